# revision 11
# baseline (speedup 1.0000x reference)
# Distributed causal self-attention for 8 Trainium2 NeuronCores (v2).
#
# Problem: B=2, T=2048, C=768, H=12 heads, D=64. y = proj(attn(qkv(x))).
#
# Sharding: 2 (batch) x 4 (head-groups of 3 heads). Core c handles batch
# c//4 and heads (c%4)*3 .. +3. Host sums the 4 head-group partials per
# batch and adds b_proj.
#
# v2 changes over the 150us baseline:
#  * S-matmuls (K=64 head dim) run as PE row-packed pairs via tile_position
#    auto-derive: heads 0/1 live at partitions 0:64 / 64:128 of one q/k
#    tile; head 2 is duplicated into both partition halves (wqkt columns
#    duplicated host-side) so its S-units pair (even i, odd i). Two K=64
#    matmuls execute concurrently -> S time halves vs K-128 zero-padding.
#  * exp splits between ScalarE (true exp) and the Vector engine using the
#    Schraudolph trick: bf16 bits of exp(SCALE*s) == round_i16(s*ASCH+BSCH),
#    one tensor_scalar per tile straight from PSUM (HW-verified round-to-
#    nearest). Rel-err budget: sim says 0.010 if ALL cols approximated.
#  * softmax denominators: ones-column fused in the v tiles gives rowsum in
#    the O accumulator; normalize = DVE reciprocal[1,512] ->
#    gpsimd.partition_broadcast -> DVE multiply (no more PE broadcast MMs).
#  * pass-major loop (PW=512, 4 passes): proj tiles for pass p interleave
#    into pass p+1's S/exp/O stream -> no serial proj tail.
#  * input DMA issues spread across 5 engine queues (issue cost ~0.6us
#    each): first matmul starts ~1.5us in.
#
# Per-head v-tile windows (448 cols): h0 lhsT=vt[:,0:128]=[v0|1|0*63] ->
# O rows 0:64, rowsum row 64; h1 lhsT=vt[:,192:320]=[0*32|1|0*31|v1] ->
# rowsum row 32 (partition APs must start 32-aligned), O rows 64:128
# (partition-aligned with its pt0[64:128] slot); h2 lhsT=vt[:,320:448]=
# [v2|1|0*63].

import numpy as np

B, T, C, H, D = 2, 2048, 768, 12, 64
HPG = 3                      # heads per group
G = 4                        # head groups
CPG = HPG * D                # 192 channels per group
KT = C // 128                # 6 contraction tiles for projections
NT = T // 128                # 16 seq tiles
PW = 512                     # tq pass width
NP = T // PW                 # 4 passes
SCALE = float(1.0 / np.sqrt(2.0))   # 1/sqrt(B) (faithful to reference)
ASCH = float(SCALE * 128.0 * np.log2(np.e))
BSCH = float(127 * 128 - 7.4)

_CACHE = {}


def _build_module():
    import concourse.bass as bass
    import concourse.tile as tile
    import concourse.mybir as mybir
    from concourse.bacc import Bacc
    from contextlib import ExitStack

    f32 = mybir.dt.float32
    bf16 = mybir.dt.bfloat16
    i16 = mybir.dt.int16
    AF = mybir.ActivationFunctionType
    MUL = mybir.AluOpType.mult
    ADD = mybir.AluOpType.add

    nc = Bacc()

    xt_d = nc.dram_tensor("xt", [C, T], bf16, kind="ExternalInput")
    wqkt_d = nc.dram_tensor("wqkt", [C, 512], bf16, kind="ExternalInput")
    wvt_d = nc.dram_tensor("wvt", [C, CPG], bf16, kind="ExternalInput")
    bqk_d = nc.dram_tensor("bqk", [128, 4], f32, kind="ExternalInput")
    bv_d = nc.dram_tensor("bv", [128, CPG], f32, kind="ExternalInput")
    wpt_d = nc.dram_tensor("wpt", [CPG, C], bf16, kind="ExternalInput")
    mask_d = nc.dram_tensor("mask", [128, 128], bf16, kind="ExternalInput")
    y_d = nc.dram_tensor("y", [T, C], bf16, kind="ExternalOutput")

    with tile.TileContext(nc) as tc, ExitStack() as ctx:
        sb = ctx.enter_context(tc.tile_pool(name="sb", bufs=1))
        ps = ctx.enter_context(tc.tile_pool(name="ps", bufs=1, space="PSUM"))

        # ---- SBUF tiles ----
        wqkt_sb = [sb.tile([128, 512], bf16, tag=f"wqk{k}", name=f"wqk{k}")
                   for k in range(KT)]
        xt_sb = [sb.tile([128, T], bf16, tag=f"xt{k}", name=f"xt{k}")
                 for k in range(KT)]
        wvt_sb = [sb.tile([128, CPG], bf16, tag=f"wv{k}", name=f"wv{k}")
                  for k in range(KT)]
        bqk_sb = sb.tile([128, 4], f32, tag="bqk", name="bqk")
        bv_sb = sb.tile([128, CPG], f32, tag="bv", name="bv")
        mask_sb = sb.tile([128, 128], bf16, tag="mask", name="mask")
        wpt0_sb = sb.tile([128, C], bf16, tag="wpt0", name="wpt0")
        wpt1_sb = sb.tile([128, C], bf16, tag="wpt1", name="wpt1")
        ones_sb = sb.tile([1, 128], bf16, tag="ones", name="ones")
        expwarm = sb.tile([1, 128], f32, tag="expwarm", name="expwarm")
        qAB = sb.tile([128, T], bf16, tag="qAB", name="qAB")
        qC = sb.tile([128, T], bf16, tag="qC", name="qC")
        kAB = sb.tile([128, T], bf16, tag="kAB", name="kAB")
        kC = sb.tile([128, T], bf16, tag="kC", name="kC")
        qk_dst = [qAB, qC, kAB, kC]
        v_sb = [sb.tile([128, 448], bf16, tag=f"v{t}", name=f"v{t}")
                for t in range(NT)]
        pt0 = sb.tile([128, T], bf16, tag="pt0", name="pt0")
        pt1 = sb.tile([128, T], bf16, tag="pt1", name="pt1")

        # ---- DMA issue plan: spread across the 3 DMA-capable queues ----
        # scalar queue: the two xt tiles gating the first matmul, then warm
        # the exp table, then two more xt tiles
        nc.scalar.dma_start(xt_sb[0][:, 0:1024], xt_d[0:128, 0:1024])
        nc.scalar.dma_start(xt_sb[1][:, 0:1024], xt_d[128:256, 0:1024])
        nc.vector.memset(ones_sb[:, :], 1.0)
        nc.scalar.activation(expwarm[:, :], ones_sb[:, :], AF.Exp)
        nc.scalar.dma_start(xt_sb[2][:, 0:1024], xt_d[256:384, 0:1024])
        nc.scalar.dma_start(xt_sb[3][:, 0:1024], xt_d[384:512, 0:1024])
        # gpsimd queue: last two xt tiles, then the memset backlog
        nc.gpsimd.dma_start(xt_sb[4][:, 0:1024], xt_d[512:640, 0:1024])
        nc.gpsimd.dma_start(xt_sb[5][:, 0:1024], xt_d[640:768, 0:1024])
        # sync queue: everything else, in need-order
        for k in range(KT):
            nc.sync.dma_start(wqkt_sb[k][:, :], wqkt_d[k * 128:(k + 1) * 128, :])
        nc.sync.dma_start(bqk_sb[:, :], bqk_d[:, :])
        nc.sync.dma_start(bv_sb[:, :], bv_d[:, :])
        nc.sync.dma_start(mask_sb[:, :], mask_d[:, :])
        for k in range(KT):
            nc.sync.dma_start(wvt_sb[k][:, :], wvt_d[k * 128:(k + 1) * 128, :])
        for k in range(KT):
            nc.sync.dma_start(xt_sb[k][:, 1024:2048],
                              xt_d[k * 128:(k + 1) * 128, 1024:2048])
        nc.sync.dma_start(wpt0_sb[:, :], wpt_d[0:128, :])
        nc.sync.dma_start(wpt1_sb[0:64, :], wpt_d[128:CPG, :])

        # gpsimd memset backlog (runs during the DMA/qkv head)
        nc.gpsimd.memset(wpt1_sb[64:128, :], 0.0)
        nc.gpsimd.memset(pt1[64:128, :], 0.0)
        for t in range(NT):
            nc.gpsimd.memset(v_sb[t][:, 65:255], 0.0)
            nc.gpsimd.memset(v_sb[t][:, 385:448], 0.0)
        for t in range(NT):
            for c1 in (64, 224, 384):
                nc.vector.memset(v_sb[t][:, c1:c1 + 1], 1.0)

        # ---- QKV projection ----
        def qk_unit(m, c, eng):
            pq = ps.tile([128, 1024], f32, tag="sp", bufs=2, name=f"pq{m}_{c}")
            for b0 in (0, 512):
                for k in range(KT):
                    nc.tensor.matmul(
                        pq[:, b0:b0 + 512],
                        lhsT=wqkt_sb[k][:, m * 128:(m + 1) * 128],
                        rhs=xt_sb[k][:, c + b0:c + b0 + 512],
                        start=(k == 0), stop=(k == KT - 1),
                    )
            dst = qk_dst[m]
            if eng == "v":
                nc.vector.tensor_scalar_add(dst[:, c:c + 1024], pq[:, :],
                                            bqk_sb[:, m:m + 1])
            else:
                nc.scalar.activation(dst[:, c:c + 1024], pq[:, :],
                                     AF.Identity, bias=bqk_sb[:, m:m + 1])

        def v_unit(t):
            pv = ps.tile([128, 512], f32, tag="ot", bufs=4, name=f"pv{t}")
            for k in range(KT):
                nc.tensor.matmul(
                    pv[:, 0:CPG],
                    lhsT=xt_sb[k][:, t * 128:(t + 1) * 128],
                    rhs=wvt_sb[k][:, :],
                    start=(k == 0), stop=(k == KT - 1),
                )
            vt = v_sb[t]
            nc.vector.tensor_add(vt[:, 0:64], pv[:, 0:64], bv_sb[:, 0:64])
            nc.vector.tensor_add(vt[:, 256:384], pv[:, 64:192],
                                 bv_sb[:, 64:192])

        qkv_order = [("q", 0, 0, "v"), ("q", 2, 0, "s"), ("v", 0), ("v", 1),
                     ("q", 1, 0, "v"), ("v", 2), ("v", 3),
                     ("q", 3, 0, "s"), ("v", 4), ("v", 5),
                     ("q", 0, 1024, "v"), ("v", 6), ("v", 7),
                     ("q", 2, 1024, "s"), ("v", 8), ("v", 9),
                     ("q", 1, 1024, "v"), ("v", 10), ("v", 11),
                     ("q", 3, 1024, "s"), ("v", 12), ("v", 13),
                     ("v", 14), ("v", 15)]
        for u in qkv_order:
            if u[0] == "q":
                qk_unit(u[1], u[2], u[3])
            else:
                v_unit(u[1])

        # ---- attention: pass-major, S row-packed pairs ----
        # unit list: per pass p: h01 pairs i=0..4p+3, then h2 pairs
        # (2j, 2j+1). Software pipeline: S(n) | exp(n-1) | O(n-2).
        units = []
        for p in range(NP):
            imax = 4 * (p + 1)
            for i in range(imax):
                units.append(("s01", p, i))
            for j in range(imax // 2):
                units.append(("s2", p, j))
        NU = len(units)                      # 60

        sp_store = {}
        ex_store = {}
        ot_store = {}
        WOFF = [0, 192, 320]                 # v-tile window starts per head
        RSROW = [64, 32, 64]                 # rowsum row per head
        OROW = [0, 64, 0]                    # O partition start per head
        PDST = [(0, 0), (0, 64), (1, 0)]     # (pt index, partition) per head

        def emit_S(n):
            kind, p, a = units[n]
            base = p * PW
            sp = ps.tile([128, 1024], f32, tag="sp", bufs=2, name=f"sp{n}")
            if kind == "s01":
                i = a
                lo = max(i * 128, base)
                for half in (0, 1):
                    r0, r1 = half * 64, half * 64 + 64
                    nc.tensor.matmul(
                        sp[:, half * 512 + lo - base:half * 512 + 512],
                        lhsT=kAB[r0:r1, i * 128:(i + 1) * 128],
                        rhs=qAB[r0:r1, lo:base + PW],
                        start=True, stop=True,
                    )
                sp_store[n] = (sp, lo - base)
            else:
                j = a
                i0, i1 = 2 * j, 2 * j + 1
                lo0 = max(i0 * 128, base)
                lo1 = max(i1 * 128, base)
                for half, (ii, lo) in enumerate(((i0, lo0), (i1, lo1))):
                    r0, r1 = half * 64, half * 64 + 64
                    nc.tensor.matmul(
                        sp[:, half * 512 + lo - base:half * 512 + 512],
                        lhsT=kC[r0:r1, ii * 128:(ii + 1) * 128],
                        rhs=qC[r0:r1, lo:base + PW],
                        start=True, stop=True,
                    )
                sp_store[n] = (sp, lo0 - base)

        def emit_E(n):
            kind, p, a = units[n]
            base = p * PW
            sp, off = sp_store.pop(n)
            ex = sb.tile([128, 1024], bf16, tag="ex", bufs=4, name=f"ex{n}")
            if n % 2 == 0:
                nc.scalar.activation(ex[:, off:1024], sp[:, off:1024],
                                     AF.Exp, scale=SCALE)
            else:
                exi = ex.bitcast(i16)
                nc.vector.tensor_scalar(exi[:, off:1024], sp[:, off:1024],
                                        ASCH, BSCH, MUL, ADD)
            # causal masks on the diagonal 128-blocks (gpsimd)
            if kind == "s01":
                i = a
                if i * 128 >= base:
                    r = i * 128 - base
                    for half in (0, 1):
                        c0 = half * 512 + r
                        nc.gpsimd.tensor_mul(ex[:, c0:c0 + 128],
                                             ex[:, c0:c0 + 128], mask_sb[:, :])
            else:
                for half, ii in enumerate((2 * a, 2 * a + 1)):
                    if ii * 128 >= base:
                        r = ii * 128 - base
                        c0 = half * 512 + r
                        nc.gpsimd.tensor_mul(ex[:, c0:c0 + 128],
                                             ex[:, c0:c0 + 128], mask_sb[:, :])
            ex_store[n] = (ex, off)

        def norm(h, p, otx):
            base = p * PW
            rs, orow = RSROW[h], OROW[h]
            pti, prow = PDST[h]
            pdst = pt0 if pti == 0 else pt1
            # rowsum row -> SBUF (ScalarE), PE ones-matmul broadcasts it to
            # all partitions, reciprocal + multiply on DVE
            rsb = sb.tile([1, 512], bf16, tag="rsb", bufs=2, name=f"rsb{h}_{p}")
            nc.scalar.activation(rsb[:, :], otx[rs:rs + 1, :], AF.Identity)
            bs = ps.tile([128, 512], f32, tag="ot", bufs=4, name=f"bs{h}_{p}")
            nc.tensor.matmul(bs[:, :], lhsT=ones_sb[:, :], rhs=rsb[:, :],
                             start=True, stop=True)
            rb = sb.tile([64, 512], f32, tag="rb", bufs=2, name=f"rb{h}_{p}")
            nc.vector.reciprocal_approx_fast(rb[:, :], bs[orow:orow + 64, :])
            nc.vector.tensor_mul(pdst[prow:prow + 64, base:base + PW],
                                 otx[orow:orow + 64, :], rb[:, :])

        def get_ot(h, p):
            if (h, p) not in ot_store:
                ot_store[(h, p)] = ps.tile([128, 512], f32, tag="ot", bufs=4,
                                           name=f"ot{h}_{p}")
            return ot_store[(h, p)]

        def emit_O(n):
            kind, p, a = units[n]
            base = p * PW
            imax = 4 * (p + 1)
            ex, off = ex_store.pop(n)
            if kind == "s01":
                i = a
                lo = max(i * 128, base)
                for half in (0, 1):
                    otx = get_ot(half, p)
                    nc.tensor.matmul(
                        otx[:, lo - base:PW],
                        lhsT=v_sb[i][:, WOFF[half]:WOFF[half] + 128],
                        rhs=ex[:, half * 512 + lo - base:half * 512 + 512],
                        start=(i == 0), stop=(i == imax - 1),
                    )
                if i == imax - 1:
                    norm(0, p, get_ot(0, p))
                    norm(1, p, get_ot(1, p))
                    ot_store.pop((0, p))
                    ot_store.pop((1, p))
            else:
                j = a
                otx = get_ot(2, p)
                for half, ii in enumerate((2 * j, 2 * j + 1)):
                    lo = max(ii * 128, base)
                    nc.tensor.matmul(
                        otx[:, lo - base:PW],
                        lhsT=v_sb[ii][:, WOFF[2]:WOFF[2] + 128],
                        rhs=ex[:, half * 512 + lo - base:half * 512 + 512],
                        start=(ii == 0), stop=(ii == imax - 1),
                    )
                if 2 * j + 1 == imax - 1:
                    norm(2, p, otx)
                    ot_store.pop((2, p))

        def proj_unit(t):
            yp = ps.tile([128, 1024], f32, tag="sp", bufs=2, name=f"yp{t}")
            for n0, nn in ((0, 512), (512, 256)):
                nc.tensor.matmul(yp[:, n0:n0 + nn],
                                 lhsT=pt0[:, t * 128:(t + 1) * 128],
                                 rhs=wpt0_sb[:, n0:n0 + nn],
                                 start=True, stop=False)
                nc.tensor.matmul(yp[:, n0:n0 + nn],
                                 lhsT=pt1[:, t * 128:(t + 1) * 128],
                                 rhs=wpt1_sb[:, n0:n0 + nn],
                                 start=False, stop=True)
            ysb = sb.tile([128, C], bf16, tag="ysb", bufs=2, name=f"ysb{t}")
            if t % 2 == 0:
                nc.scalar.activation(ysb[:, 0:C], yp[:, 0:C], AF.Identity)
            else:
                nc.vector.tensor_copy(ysb[:, 0:C], yp[:, 0:C])
            nc.sync.dma_start(y_d[t * 128:(t + 1) * 128, :], ysb[:, 0:C])

        # pass-end unit indices and proj injection bookkeeping
        pass_end = []
        acc = 0
        for p in range(NP):
            acc += 4 * (p + 1) + 2 * (p + 1)
            pass_end.append(acc - 1)
        proj_queue = []
        proj_ready_at = {pass_end[p] + 3: p for p in range(NP)}
        enqueued = set()

        for n in range(NU + 2):
            if n in proj_ready_at:
                p = proj_ready_at[n]
                proj_queue.extend(range(4 * p, 4 * p + 4))
                enqueued.add(p)
            if n < NU:
                emit_S(n)
            if 1 <= n <= NU:
                emit_E(n - 1)
            if n >= 2:
                emit_O(n - 2)
            # inject at most one proj tile per iteration, spaced out
            if proj_queue and n % 2 == 1:
                proj_unit(proj_queue.pop(0))
        for p in range(NP):
            if p not in enqueued:
                proj_queue.extend(range(4 * p, 4 * p + 4))
        while proj_queue:
            proj_unit(proj_queue.pop(0))

    nc.finalize()
    return nc


def _get_module():
    if "nc" not in _CACHE:
        _CACHE["nc"] = _build_module()
    return _CACHE["nc"]


def make_in_maps(x, w_attn, b_attn, w_proj):
    """Host-side sharding: per-core input dicts (8 cores)."""
    import ml_dtypes
    bf16 = ml_dtypes.bfloat16
    x = np.asarray(x, dtype=np.float32)
    w_attn = np.asarray(w_attn, dtype=np.float32)
    b_attn = np.asarray(b_attn, dtype=np.float32)
    w_proj = np.asarray(w_proj, dtype=np.float32)

    xts = [np.ascontiguousarray(x[b].T).astype(bf16) for b in range(B)]
    mask = np.triu(np.ones((128, 128), np.float32)).astype(bf16)

    in_maps = []
    for c in range(8):
        b = c // G
        hg = c % G
        sl = slice(CPG * hg, CPG * (hg + 1))
        wq = w_attn[0:C][sl]          # [192, 768]
        wk = w_attn[C:2 * C][sl]
        wv = w_attn[2 * C:3 * C][sl]
        # m-tiles: [q01 | q2,q2 | k01 | k2,k2]  -> [768, 512]
        wqkt = np.concatenate([
            wq[0:128], wq[128:192], wq[128:192],
            wk[0:128], wk[128:192], wk[128:192],
        ], axis=0).T
        wqkt = np.ascontiguousarray(wqkt).astype(bf16)
        wvt = np.ascontiguousarray(wv.T).astype(bf16)             # [768, 192]
        bq = b_attn[0:C][sl]
        bk = b_attn[C:2 * C][sl]
        bv = b_attn[2 * C:3 * C][sl]
        bqk = np.zeros((128, 4), np.float32)
        bqk[:, 0] = bq[0:128]
        bqk[:, 1] = np.concatenate([bq[128:192], bq[128:192]])
        bqk[:, 2] = bk[0:128]
        bqk[:, 3] = np.concatenate([bk[128:192], bk[128:192]])
        bvb = np.ascontiguousarray(
            np.broadcast_to(bv, (128, CPG))).astype(np.float32)   # [128, 192]
        wpt = np.ascontiguousarray(w_proj[:, sl].T).astype(bf16)  # [192, 768]
        in_maps.append({
            "xt": xts[b],
            "wqkt": wqkt,
            "wvt": wvt,
            "bqk": bqk,
            "bv": bvb,
            "wpt": wpt,
            "mask": mask,
        })
    return in_maps


def gather(results, b_proj):
    """Sum the 4 head-group partials per batch, add bias."""
    b_proj = np.asarray(b_proj, dtype=np.float32)
    y = np.zeros((B, T, C), np.float32)
    for c in range(8):
        y[c // G] += np.asarray(results[c]["y"], dtype=np.float32)
    y += b_proj
    return y


def run(x, w_attn, b_attn, w_proj, b_proj, trace=False, **kw):
    from concourse.bass_utils import run_bass_kernel_spmd
    nc = _get_module()
    in_maps = make_in_maps(x, w_attn, b_attn, w_proj)
    res = run_bass_kernel_spmd(nc, in_maps, list(range(8)), trace=trace, **kw)
    return gather(res.results, b_proj), res


def kernel(x, w_attn, b_attn, w_proj, b_proj):
    y, _ = run(x, w_attn, b_attn, w_proj, b_proj)
    return y


# revision 19
# speedup vs baseline: 1.2158x; 1.2158x over previous
# Distributed causal self-attention for 8 Trainium2 NeuronCores (v2).
#
# Problem: B=2, T=2048, C=768, H=12 heads, D=64. y = proj(attn(qkv(x))).
#
# Sharding: 2 (batch) x 4 (head-groups of 3 heads). Core c handles batch
# c//4 and heads (c%4)*3 .. +3. Host sums the 4 head-group partials per
# batch and adds b_proj.
#
# v2 changes over the 150us baseline:
#  * S-matmuls (K=64 head dim) run as PE row-packed pairs via tile_position
#    auto-derive: heads 0/1 live at partitions 0:64 / 64:128 of one q/k
#    tile; head 2 is duplicated into both partition halves (wqkt columns
#    duplicated host-side) so its S-units pair (even i, odd i). Two K=64
#    matmuls execute concurrently -> S time halves vs K-128 zero-padding.
#  * exp splits between ScalarE (true exp) and the Vector engine using the
#    Schraudolph trick: bf16 bits of exp(SCALE*s) == round_i16(s*ASCH+BSCH),
#    one tensor_scalar per tile straight from PSUM (HW-verified round-to-
#    nearest). Rel-err budget: sim says 0.010 if ALL cols approximated.
#  * softmax denominators: ones-column fused in the v tiles gives rowsum in
#    the O accumulator; normalize = DVE reciprocal[1,512] ->
#    gpsimd.partition_broadcast -> DVE multiply (no more PE broadcast MMs).
#  * pass-major loop (PW=512, 4 passes): proj tiles for pass p interleave
#    into pass p+1's S/exp/O stream -> no serial proj tail.
#  * input DMA issues spread across 5 engine queues (issue cost ~0.6us
#    each): first matmul starts ~1.5us in.
#
# Per-head v-tile windows (448 cols): h0 lhsT=vt[:,0:128]=[v0|1|0*63] ->
# O rows 0:64, rowsum row 64; h1 lhsT=vt[:,192:320]=[0*32|1|0*31|v1] ->
# rowsum row 32 (partition APs must start 32-aligned), O rows 64:128
# (partition-aligned with its pt0[64:128] slot); h2 lhsT=vt[:,320:448]=
# [v2|1|0*63].

import numpy as np

B, T, C, H, D = 2, 2048, 768, 12, 64
HPG = 3                      # heads per group
G = 4                        # head groups
CPG = HPG * D                # 192 channels per group
KT = C // 128                # 6 contraction tiles for projections
NT = T // 128                # 16 seq tiles
PW = 512                     # tq pass width
NP = T // PW                 # 4 passes
SCALE = float(1.0 / np.sqrt(2.0))   # 1/sqrt(B) (faithful to reference)
ASCH = float(SCALE * 128.0 * np.log2(np.e))
BSCH = float(127 * 128 - 7.4)

_CACHE = {}


def _build_module():
    import concourse.bass as bass
    import concourse.tile as tile
    import concourse.mybir as mybir
    from concourse.bacc import Bacc
    from contextlib import ExitStack

    f32 = mybir.dt.float32
    bf16 = mybir.dt.bfloat16
    i16 = mybir.dt.int16
    AF = mybir.ActivationFunctionType
    MUL = mybir.AluOpType.mult
    ADD = mybir.AluOpType.add

    nc = Bacc()

    xt_d = nc.dram_tensor("xt", [C, T], bf16, kind="ExternalInput")
    wqkt_d = nc.dram_tensor("wqkt", [C, 512], bf16, kind="ExternalInput")
    wvt_d = nc.dram_tensor("wvt", [C, CPG], bf16, kind="ExternalInput")
    bqk_d = nc.dram_tensor("bqk", [128, 4], f32, kind="ExternalInput")
    bv_d = nc.dram_tensor("bv", [128, CPG], f32, kind="ExternalInput")
    wpt_d = nc.dram_tensor("wpt", [CPG, C], bf16, kind="ExternalInput")
    mask_d = nc.dram_tensor("mask", [128, 128], bf16, kind="ExternalInput")
    y_d = nc.dram_tensor("y", [T, C], bf16, kind="ExternalOutput")

    with tile.TileContext(nc) as tc, ExitStack() as ctx:
        sb = ctx.enter_context(tc.tile_pool(name="sb", bufs=1))
        ps = ctx.enter_context(tc.tile_pool(name="ps", bufs=1, space="PSUM"))

        # ---- SBUF tiles ----
        wqkt_sb = [sb.tile([128, 512], bf16, tag=f"wqk{k}", name=f"wqk{k}")
                   for k in range(KT)]
        xt_sb = [sb.tile([128, T], bf16, tag=f"xt{k}", name=f"xt{k}")
                 for k in range(KT)]
        wvt_sb = [sb.tile([128, CPG], bf16, tag=f"wv{k}", name=f"wv{k}")
                  for k in range(KT)]
        bqk_sb = sb.tile([128, 4], f32, tag="bqk", name="bqk")
        bv_sb = sb.tile([128, CPG], f32, tag="bv", name="bv")
        mask_sb = sb.tile([128, 128], bf16, tag="mask", name="mask")
        wpt0_sb = sb.tile([128, C], bf16, tag="wpt0", name="wpt0")
        wpt1_sb = sb.tile([128, C], bf16, tag="wpt1", name="wpt1")
        ones_sb = sb.tile([1, 128], bf16, tag="ones", name="ones")
        expwarm = sb.tile([1, 128], f32, tag="expwarm", name="expwarm")
        qAB = sb.tile([128, T], bf16, tag="qAB", name="qAB")
        qC = sb.tile([128, T], bf16, tag="qC", name="qC")
        kAB = sb.tile([128, T], bf16, tag="kAB", name="kAB")
        kC = sb.tile([128, T], bf16, tag="kC", name="kC")
        qk_dst = [qAB, qC, kAB, kC]
        v_sb = [sb.tile([128, 448], bf16, tag=f"v{t}", name=f"v{t}")
                for t in range(NT)]
        pt0 = sb.tile([128, T], bf16, tag="pt0", name="pt0")
        pt1 = sb.tile([128, T], bf16, tag="pt1", name="pt1")

        # ---- DMA issue plan: spread across the 3 DMA-capable queues ----
        # scalar queue: the two xt tiles gating the first matmul, then warm
        # the exp table, then two more xt tiles
        nc.scalar.dma_start(xt_sb[0][:, 0:1024], xt_d[0:128, 0:1024])
        nc.scalar.dma_start(xt_sb[1][:, 0:1024], xt_d[128:256, 0:1024])
        nc.vector.memset(ones_sb[:, :], 1.0)
        nc.scalar.activation(expwarm[:, :], ones_sb[:, :], AF.Exp)
        nc.scalar.dma_start(xt_sb[2][:, 0:1024], xt_d[256:384, 0:1024])
        nc.scalar.dma_start(xt_sb[3][:, 0:1024], xt_d[384:512, 0:1024])
        # gpsimd queue: last two xt tiles, then the memset backlog
        nc.gpsimd.dma_start(xt_sb[4][:, 0:1024], xt_d[512:640, 0:1024])
        nc.gpsimd.dma_start(xt_sb[5][:, 0:1024], xt_d[640:768, 0:1024])
        # sync queue: everything else, in need-order
        for k in range(KT):
            nc.sync.dma_start(wqkt_sb[k][:, :], wqkt_d[k * 128:(k + 1) * 128, :])
        nc.sync.dma_start(bqk_sb[:, :], bqk_d[:, :])
        nc.sync.dma_start(bv_sb[:, :], bv_d[:, :])
        nc.sync.dma_start(mask_sb[:, :], mask_d[:, :])
        for k in range(KT):
            nc.sync.dma_start(wvt_sb[k][:, :], wvt_d[k * 128:(k + 1) * 128, :])
        for k in range(KT):
            nc.sync.dma_start(xt_sb[k][:, 1024:2048],
                              xt_d[k * 128:(k + 1) * 128, 1024:2048])
        nc.sync.dma_start(wpt0_sb[:, :], wpt_d[0:128, :])
        nc.sync.dma_start(wpt1_sb[0:64, :], wpt_d[128:CPG, :])

        # gpsimd memset backlog (runs during the DMA/qkv head)
        nc.gpsimd.memset(wpt1_sb[64:128, :], 0.0)
        nc.gpsimd.memset(pt1[64:128, :], 0.0)
        for t in range(NT):
            nc.gpsimd.memset(v_sb[t][:, 65:256], 0.0)
            nc.gpsimd.memset(v_sb[t][:, 385:448], 0.0)
        for t in range(NT):
            for c1 in (64, 224, 384):
                nc.vector.memset(v_sb[t][:, c1:c1 + 1], 1.0)

        # ---- QKV projection ----
        # pq in 1-bank [128,512] halves so the sp pool (bufs=5) keeps the
        # PE pipeline deep from the very first matmul (HAM warms early)
        def qk_unit(m, c, eng):
            dst = qk_dst[m]
            pq = ps.tile([128, 512], f32, tag="sp", bufs=5,
                         name=f"pq{m}_{c}")
            for k in range(KT):
                nc.tensor.matmul(
                    pq[:, :],
                    lhsT=wqkt_sb[k][:, m * 128:(m + 1) * 128],
                    rhs=xt_sb[k][:, c:c + 512],
                    start=(k == 0), stop=(k == KT - 1),
                )
            if eng == "v":
                nc.vector.tensor_scalar_add(dst[:, c:c + 512], pq[:, :],
                                            bqk_sb[:, m:m + 1])
            else:
                nc.scalar.activation(dst[:, c:c + 512], pq[:, :],
                                     AF.Identity, bias=bqk_sb[:, m:m + 1])

        def v_unit(t):
            pv = ps.tile([128, 512], f32, tag="ot", bufs=3, name=f"pv{t}")
            for k in range(KT):
                nc.tensor.matmul(
                    pv[:, 0:CPG],
                    lhsT=xt_sb[k][:, t * 128:(t + 1) * 128],
                    rhs=wvt_sb[k][:, :],
                    start=(k == 0), stop=(k == KT - 1),
                )
            vt = v_sb[t]
            nc.vector.tensor_add(vt[:, 0:64], pv[:, 0:64], bv_sb[:, 0:64])
            nc.vector.tensor_add(vt[:, 256:384], pv[:, 64:192],
                                 bv_sb[:, 64:192])

        # all of qkv+v runs before attention; S/exp work only starts after,
        # so the ot slots are never contended by pv during the passes
        qkv_order = []
        for c in (0, 512, 1024, 1536):
            for m in (0, 2, 1, 3):
                qkv_order.append(("q", m, c, "s" if (m + c // 512) % 2 else "v"))
        vq = list(range(NT))
        mixed = []
        for j, u in enumerate(qkv_order):
            mixed.append(u)
            if j % 2 == 1 and vq:
                mixed.append(("v", vq.pop(0)))
            if j >= 8 and vq:
                mixed.append(("v", vq.pop(0)))
        for u in mixed:
            if u[0] == "q":
                qk_unit(u[1], u[2], u[3])
            else:
                v_unit(u[1])
        while vq:
            v_unit(vq.pop(0))

        # ---- attention: pass-major, S row-packed pairs ----
        # unit list: per pass p: h01 pairs i=0..4p+3, then h2 pairs
        # (2j, 2j+1). Software pipeline: S(n) | exp(n-1) | O(n-2).
        units = []
        for p in range(NP):
            imax = 4 * (p + 1)
            for i in range(imax):
                units.append(("s01", p, i))
            for j in range(imax // 2):
                units.append(("s2", p, j))
        NU = len(units)                      # 60

        sp_store = {}
        ex_store = {}
        ot_store = {}
        WOFF = [0, 192, 320]                 # v-tile window starts per head
        RSROW = [64, 32, 64]                 # rowsum row per head
        OROW = [0, 64, 0]                    # O partition start per head
        PDST = [(0, 0), (0, 64), (1, 0)]     # (pt index, partition) per head

        def emit_S(n):
            kind, p, a = units[n]
            base = p * PW
            spA = ps.tile([128, 512], f32, tag="sp", bufs=5, name=f"spA{n}")
            spB = ps.tile([128, 512], f32, tag="sp", bufs=5, name=f"spB{n}")
            if kind == "s01":
                i0 = i1 = a
                kt, qt = kAB, qAB
            else:
                i0, i1 = 2 * a, 2 * a + 1
                kt, qt = kC, qC
            lo0 = max(i0 * 128, base)
            lo1 = max(i1 * 128, base)
            for half, (sp, ii, lo) in enumerate(((spA, i0, lo0),
                                                 (spB, i1, lo1))):
                r0, r1 = half * 64, half * 64 + 64
                nc.tensor.matmul(
                    sp[:, lo - base:512],
                    lhsT=kt[r0:r1, ii * 128:(ii + 1) * 128],
                    rhs=qt[r0:r1, lo:base + PW],
                    start=True, stop=True,
                )
            sp_store[n] = (spA, spB, lo0 - base, lo1 - base)

        def emit_E(n):
            kind, p, a = units[n]
            base = p * PW
            spA, spB, offA, offB = sp_store.pop(n)
            ex = sb.tile([128, 1024], bf16, tag="ex", bufs=4, name=f"ex{n}")
            exi = ex.bitcast(i16)
            # the two halves run concurrently on ScalarE and DVE
            if n % 2 == 0:
                nc.scalar.activation(ex[:, offA:512], spA[:, offA:512],
                                     AF.Exp, scale=SCALE)
                nc.vector.tensor_scalar(exi[:, 512 + offB:1024],
                                        spB[:, offB:512], ASCH, BSCH, MUL, ADD)
            else:
                nc.vector.tensor_scalar(exi[:, offA:512], spA[:, offA:512],
                                        ASCH, BSCH, MUL, ADD)
                nc.scalar.activation(ex[:, 512 + offB:1024], spB[:, offB:512],
                                     AF.Exp, scale=SCALE)
            # causal masks on the diagonal 128-blocks (gpsimd)
            i0 = a if kind == "s01" else 2 * a
            if i0 * 128 >= base:
                r = i0 * 128 - base
                shift = 512 if kind == "s01" else 640
                for c0 in (r, r + shift):
                    nc.gpsimd.tensor_mul(ex[:, c0:c0 + 128],
                                         ex[:, c0:c0 + 128], mask_sb[:, :])
            ex_store[n] = (ex, offA, offB)

        def norm(h, p, otx):
            base = p * PW
            rs, orow = RSROW[h], OROW[h]
            pti, prow = PDST[h]
            pdst = pt0 if pti == 0 else pt1
            # rowsum row -> SBUF (ScalarE); PE ones-matmul broadcasts it into
            # a transient 1-bank sp slot; reciprocal + multiply on DVE
            rsb = sb.tile([1, 512], bf16, tag="rsb", bufs=2, name=f"rsb{h}_{p}")
            nc.scalar.activation(rsb[:, :], otx[rs:rs + 1, :], AF.Identity)
            bs = ps.tile([128, 512], f32, tag="sp", bufs=5, name=f"bs{h}_{p}")
            nc.tensor.matmul(bs[:, :], lhsT=ones_sb[:, :], rhs=rsb[:, :],
                             start=True, stop=True)
            rb = sb.tile([64, 512], f32, tag="rb", bufs=2, name=f"rb{h}_{p}")
            nc.vector.reciprocal_approx_fast(rb[:, :], bs[orow:orow + 64, :])
            nc.vector.tensor_mul(pdst[prow:prow + 64, base:base + PW],
                                 otx[orow:orow + 64, :], rb[:, :])

        def get_ot(h, p):
            if (h, p) not in ot_store:
                ot_store[(h, p)] = ps.tile([128, 512], f32, tag="ot", bufs=3,
                                           name=f"ot{h}_{p}")
            return ot_store[(h, p)]

        def emit_O(n):
            kind, p, a = units[n]
            base = p * PW
            imax = 4 * (p + 1)
            ex, offA, offB = ex_store.pop(n)
            if kind == "s01":
                i = a
                lo = max(i * 128, base)
                for half in (0, 1):
                    otx = get_ot(half, p)
                    nc.tensor.matmul(
                        otx[:, lo - base:PW],
                        lhsT=v_sb[i][:, WOFF[half]:WOFF[half] + 128],
                        rhs=ex[:, half * 512 + lo - base:half * 512 + 512],
                        start=(i == 0), stop=(i == imax - 1),
                    )
                if i == imax - 1:
                    norm(0, p, get_ot(0, p))
                    norm(1, p, get_ot(1, p))
                    ot_store.pop((0, p))
                    ot_store.pop((1, p))
            else:
                j = a
                otx = get_ot(2, p)
                for half, ii in enumerate((2 * j, 2 * j + 1)):
                    lo = max(ii * 128, base)
                    nc.tensor.matmul(
                        otx[:, lo - base:PW],
                        lhsT=v_sb[ii][:, WOFF[2]:WOFF[2] + 128],
                        rhs=ex[:, half * 512 + lo - base:half * 512 + 512],
                        start=(ii == 0), stop=(ii == imax - 1),
                    )
                if 2 * j + 1 == imax - 1:
                    norm(2, p, otx)
                    ot_store.pop((2, p))

        PROJ_EV = [("s", 0, 384), ("v", 384, 384)]

        def proj_unit(t):
            ysb = sb.tile([128, C], bf16, tag="ysb", bufs=2, name=f"ysb{t}")
            for eng, n0, nn in PROJ_EV:
                yp = ps.tile([128, 512], f32, tag="sp", bufs=5,
                             name=f"yp{t}_{n0}")
                nc.tensor.matmul(yp[:, 0:nn],
                                 lhsT=pt0[:, t * 128:(t + 1) * 128],
                                 rhs=wpt0_sb[:, n0:n0 + nn],
                                 start=True, stop=False)
                nc.tensor.matmul(yp[:, 0:nn],
                                 lhsT=pt1[:, t * 128:(t + 1) * 128],
                                 rhs=wpt1_sb[:, n0:n0 + nn],
                                 start=False, stop=True)
                if eng == "s":
                    nc.scalar.activation(ysb[:, n0:n0 + nn], yp[:, 0:nn],
                                         AF.Identity)
                else:
                    nc.vector.tensor_copy(ysb[:, n0:n0 + nn], yp[:, 0:nn])
            nc.sync.dma_start(y_d[t * 128:(t + 1) * 128, :], ysb[:, 0:C])

        # pass-end unit indices and proj injection bookkeeping
        pass_end = []
        acc = 0
        for p in range(NP):
            acc += 4 * (p + 1) + 2 * (p + 1)
            pass_end.append(acc - 1)
        proj_queue = []
        proj_ready_at = {pass_end[p] + 3: p for p in range(NP)}
        enqueued = set()

        for n in range(NU + 2):
            if n in proj_ready_at:
                p = proj_ready_at[n]
                proj_queue.extend(range(4 * p, 4 * p + 4))
                enqueued.add(p)
            if n < NU:
                emit_S(n)
            if 1 <= n <= NU:
                emit_E(n - 1)
            if n >= 2:
                emit_O(n - 2)
            # inject at most one proj tile per iteration, spaced out
            if proj_queue and n % 2 == 1:
                proj_unit(proj_queue.pop(0))
        for p in range(NP):
            if p not in enqueued:
                proj_queue.extend(range(4 * p, 4 * p + 4))
        while proj_queue:
            proj_unit(proj_queue.pop(0))

    nc.finalize()
    return nc


def _get_module():
    if "nc" not in _CACHE:
        _CACHE["nc"] = _build_module()
    return _CACHE["nc"]


def make_in_maps(x, w_attn, b_attn, w_proj):
    """Host-side sharding: per-core input dicts (8 cores)."""
    import ml_dtypes
    bf16 = ml_dtypes.bfloat16
    x = np.asarray(x, dtype=np.float32)
    w_attn = np.asarray(w_attn, dtype=np.float32)
    b_attn = np.asarray(b_attn, dtype=np.float32)
    w_proj = np.asarray(w_proj, dtype=np.float32)

    xts = [np.ascontiguousarray(x[b].T).astype(bf16) for b in range(B)]
    mask = np.triu(np.ones((128, 128), np.float32)).astype(bf16)

    in_maps = []
    for c in range(8):
        b = c // G
        hg = c % G
        sl = slice(CPG * hg, CPG * (hg + 1))
        wq = w_attn[0:C][sl]          # [192, 768]
        wk = w_attn[C:2 * C][sl]
        wv = w_attn[2 * C:3 * C][sl]
        # m-tiles: [q01 | q2,q2 | k01 | k2,k2]  -> [768, 512]
        wqkt = np.concatenate([
            wq[0:128], wq[128:192], wq[128:192],
            wk[0:128], wk[128:192], wk[128:192],
        ], axis=0).T
        wqkt = np.ascontiguousarray(wqkt).astype(bf16)
        wvt = np.ascontiguousarray(wv.T).astype(bf16)             # [768, 192]
        bq = b_attn[0:C][sl]
        bk = b_attn[C:2 * C][sl]
        bv = b_attn[2 * C:3 * C][sl]
        bqk = np.zeros((128, 4), np.float32)
        bqk[:, 0] = bq[0:128]
        bqk[:, 1] = np.concatenate([bq[128:192], bq[128:192]])
        bqk[:, 2] = bk[0:128]
        bqk[:, 3] = np.concatenate([bk[128:192], bk[128:192]])
        bvb = np.ascontiguousarray(
            np.broadcast_to(bv, (128, CPG))).astype(np.float32)   # [128, 192]
        wpt = np.ascontiguousarray(w_proj[:, sl].T).astype(bf16)  # [192, 768]
        in_maps.append({
            "xt": xts[b],
            "wqkt": wqkt,
            "wvt": wvt,
            "bqk": bqk,
            "bv": bvb,
            "wpt": wpt,
            "mask": mask,
        })
    return in_maps


def gather(results, b_proj):
    """Sum the 4 head-group partials per batch, add bias."""
    b_proj = np.asarray(b_proj, dtype=np.float32)
    y = np.zeros((B, T, C), np.float32)
    for c in range(8):
        y[c // G] += np.asarray(results[c]["y"], dtype=np.float32)
    y += b_proj
    return y


def run(x, w_attn, b_attn, w_proj, b_proj, trace=False, **kw):
    from concourse.bass_utils import run_bass_kernel_spmd
    nc = _get_module()
    in_maps = make_in_maps(x, w_attn, b_attn, w_proj)
    res = run_bass_kernel_spmd(nc, in_maps, list(range(8)), trace=trace, **kw)
    return gather(res.results, b_proj), res


def kernel(x, w_attn, b_attn, w_proj, b_proj):
    y, _ = run(x, w_attn, b_attn, w_proj, b_proj)
    return y


# revision 28
# speedup vs baseline: 1.2376x; 1.0180x over previous
# Distributed causal self-attention for 8 Trainium2 NeuronCores (v2).
#
# Problem: B=2, T=2048, C=768, H=12 heads, D=64. y = proj(attn(qkv(x))).
#
# Sharding: 2 (batch) x 4 (head-groups of 3 heads). Core c handles batch
# c//4 and heads (c%4)*3 .. +3. Host sums the 4 head-group partials per
# batch and adds b_proj.
#
# v2 changes over the 150us baseline:
#  * S-matmuls (K=64 head dim) run as PE row-packed pairs via tile_position
#    auto-derive: heads 0/1 live at partitions 0:64 / 64:128 of one q/k
#    tile; head 2 is duplicated into both partition halves (wqkt columns
#    duplicated host-side) so its S-units pair (even i, odd i). Two K=64
#    matmuls execute concurrently -> S time halves vs K-128 zero-padding.
#  * exp splits between ScalarE (true exp) and the Vector engine using the
#    Schraudolph trick: bf16 bits of exp(SCALE*s) == round_i16(s*ASCH+BSCH),
#    one tensor_scalar per tile straight from PSUM (HW-verified round-to-
#    nearest). Rel-err budget: sim says 0.010 if ALL cols approximated.
#  * softmax denominators: ones-column fused in the v tiles gives rowsum in
#    the O accumulator; normalize = DVE reciprocal[1,512] ->
#    gpsimd.partition_broadcast -> DVE multiply (no more PE broadcast MMs).
#  * pass-major loop (PW=512, 4 passes): proj tiles for pass p interleave
#    into pass p+1's S/exp/O stream -> no serial proj tail.
#  * input DMA issues spread across 5 engine queues (issue cost ~0.6us
#    each): first matmul starts ~1.5us in.
#
# Per-head v-tile windows (448 cols): h0 lhsT=vt[:,0:128]=[v0|1|0*63] ->
# O rows 0:64, rowsum row 64; h1 lhsT=vt[:,192:320]=[0*32|1|0*31|v1] ->
# rowsum row 32 (partition APs must start 32-aligned), O rows 64:128
# (partition-aligned with its pt0[64:128] slot); h2 lhsT=vt[:,320:448]=
# [v2|1|0*63].

import numpy as np

B, T, C, H, D = 2, 2048, 768, 12, 64
HPG = 3                      # heads per group
G = 4                        # head groups
CPG = HPG * D                # 192 channels per group
KT = C // 128                # 6 contraction tiles for projections
NT = T // 128                # 16 seq tiles
PW = 512                     # tq pass width
NP = T // PW                 # 4 passes
SCALE = float(1.0 / np.sqrt(2.0))   # 1/sqrt(B) (faithful to reference)
ASCH = float(SCALE * 128.0 * np.log2(np.e))
BSCH = float(127 * 128 - 7.4)

_CACHE = {}


def _build_module():
    import concourse.bass as bass
    import concourse.tile as tile
    import concourse.mybir as mybir
    from concourse.bacc import Bacc
    from contextlib import ExitStack

    f32 = mybir.dt.float32
    bf16 = mybir.dt.bfloat16
    i16 = mybir.dt.int16
    AF = mybir.ActivationFunctionType
    MUL = mybir.AluOpType.mult
    ADD = mybir.AluOpType.add

    nc = Bacc()

    xt_d = nc.dram_tensor("xt", [C, T], bf16, kind="ExternalInput")
    wqkt_d = nc.dram_tensor("wqkt", [C, 512], bf16, kind="ExternalInput")
    wvt_d = nc.dram_tensor("wvt", [C, CPG], bf16, kind="ExternalInput")
    bqk_d = nc.dram_tensor("bqk", [128, 4], f32, kind="ExternalInput")
    bv_d = nc.dram_tensor("bv", [128, CPG], f32, kind="ExternalInput")
    wpt_d = nc.dram_tensor("wpt", [CPG, C], bf16, kind="ExternalInput")
    mask_d = nc.dram_tensor("mask", [128, 128], bf16, kind="ExternalInput")
    y_d = nc.dram_tensor("y", [T, C], bf16, kind="ExternalOutput")

    with tile.TileContext(nc) as tc, ExitStack() as ctx:
        sb = ctx.enter_context(tc.tile_pool(name="sb", bufs=1))
        ps = ctx.enter_context(tc.tile_pool(name="ps", bufs=1, space="PSUM"))

        # ---- SBUF tiles ----
        wqkt_sb = [sb.tile([128, 512], bf16, tag=f"wqk{k}", name=f"wqk{k}")
                   for k in range(KT)]
        xt_sb = [sb.tile([128, T], bf16, tag=f"xt{k}", name=f"xt{k}")
                 for k in range(KT)]
        wvt_sb = [sb.tile([128, CPG], bf16, tag=f"wv{k}", name=f"wv{k}")
                  for k in range(KT)]
        bqk_sb = sb.tile([128, 4], f32, tag="bqk", name="bqk")
        bv_sb = sb.tile([128, CPG], f32, tag="bv", name="bv")
        mask_sb = sb.tile([128, 128], bf16, tag="mask", name="mask")
        wpt0_sb = sb.tile([128, C], bf16, tag="wpt0", name="wpt0")
        wpt1_sb = sb.tile([128, C], bf16, tag="wpt1", name="wpt1")
        ones_sb = sb.tile([1, 128], bf16, tag="ones", name="ones")
        expwarm = sb.tile([1, 128], f32, tag="expwarm", name="expwarm")
        qAB = sb.tile([128, T], bf16, tag="qAB", name="qAB")
        qC = sb.tile([128, T], bf16, tag="qC", name="qC")
        kAB = sb.tile([128, T], bf16, tag="kAB", name="kAB")
        kC = sb.tile([128, T], bf16, tag="kC", name="kC")
        qk_dst = [qAB, qC, kAB, kC]
        v_sb = [sb.tile([128, 448], bf16, tag=f"v{t}", name=f"v{t}")
                for t in range(NT)]
        pt0 = sb.tile([128, T], bf16, tag="pt0", name="pt0")
        pt1 = sb.tile([128, T], bf16, tag="pt1", name="pt1")

        # ---- DMA issue plan: spread across the 3 DMA-capable queues ----
        # scalar queue: two xt tiles and two wqkt tiles gating the first
        # qk unit, then warm the exp table (~2.7us, must not delay DMAs)
        nc.scalar.dma_start(xt_sb[0][:, 0:1024], xt_d[0:128, 0:1024])
        nc.scalar.dma_start(xt_sb[1][:, 0:1024], xt_d[128:256, 0:1024])
        nc.vector.memset(ones_sb[:, :], 1.0)
        nc.scalar.dma_start(wqkt_sb[4][:, :], wqkt_d[512:640, :])
        nc.scalar.dma_start(wqkt_sb[5][:, :], wqkt_d[640:768, :])
        nc.scalar.activation(expwarm[:, :], ones_sb[:, :], AF.Exp)
        # gpsimd queue: last four xt tiles, then the memset backlog
        nc.gpsimd.dma_start(xt_sb[2][:, 0:1024], xt_d[256:384, 0:1024])
        nc.gpsimd.dma_start(xt_sb[3][:, 0:1024], xt_d[384:512, 0:1024])
        nc.gpsimd.dma_start(xt_sb[4][:, 0:1024], xt_d[512:640, 0:1024])
        nc.gpsimd.dma_start(xt_sb[5][:, 0:1024], xt_d[640:768, 0:1024])
        # sync queue: everything else, in need-order
        for k in range(4):
            nc.sync.dma_start(wqkt_sb[k][:, :], wqkt_d[k * 128:(k + 1) * 128, :])
        nc.sync.dma_start(bqk_sb[:, :], bqk_d[:, :])
        nc.sync.dma_start(bv_sb[:, :], bv_d[:, :])
        for k in range(KT):
            nc.sync.dma_start(wvt_sb[k][:, :], wvt_d[k * 128:(k + 1) * 128, :])
        for k in range(KT):
            nc.sync.dma_start(xt_sb[k][:, 1024:2048],
                              xt_d[k * 128:(k + 1) * 128, 1024:2048])
        nc.sync.dma_start(mask_sb[:, :], mask_d[:, :])
        nc.sync.dma_start(wpt0_sb[:, :], wpt_d[0:128, :])
        nc.sync.dma_start(wpt1_sb[0:64, :], wpt_d[128:CPG, :])

        # gpsimd memset backlog (runs during the DMA/qkv head)
        nc.gpsimd.memset(wpt1_sb[64:128, :], 0.0)
        nc.gpsimd.memset(pt1[64:128, :], 0.0)
        for t in range(NT):
            nc.gpsimd.memset(v_sb[t][:, 65:256], 0.0)
            nc.gpsimd.memset(v_sb[t][:, 385:448], 0.0)
        for t in range(NT):
            for c1 in (64, 224, 384):
                nc.vector.memset(v_sb[t][:, c1:c1 + 1], 1.0)

        # ---- QKV projection ----
        # pq in 1-bank [128,512] halves so the sp pool (bufs=5) keeps the
        # PE pipeline deep from the very first matmul (HAM warms early)
        def qk_unit(m, c, eng):
            dst = qk_dst[m]
            pq = ps.tile([128, 512], f32, tag="sp", bufs=5,
                         name=f"pq{m}_{c}")
            for k in range(KT):
                nc.tensor.matmul(
                    pq[:, :],
                    lhsT=wqkt_sb[k][:, m * 128:(m + 1) * 128],
                    rhs=xt_sb[k][:, c:c + 512],
                    start=(k == 0), stop=(k == KT - 1),
                )
            if eng == "v":
                nc.vector.tensor_scalar_add(dst[:, c:c + 512], pq[:, :],
                                            bqk_sb[:, m:m + 1])
            else:
                nc.scalar.activation(dst[:, c:c + 512], pq[:, :],
                                     AF.Identity, bias=bqk_sb[:, m:m + 1])

        def v_unit(t):
            pv = ps.tile([128, 512], f32, tag="ot", bufs=3, name=f"pv{t}")
            for k in range(KT):
                nc.tensor.matmul(
                    pv[:, 0:CPG],
                    lhsT=xt_sb[k][:, t * 128:(t + 1) * 128],
                    rhs=wvt_sb[k][:, :],
                    start=(k == 0), stop=(k == KT - 1),
                )
            vt = v_sb[t]
            nc.vector.tensor_add(vt[:, 0:64], pv[:, 0:64], bv_sb[:, 0:64])
            nc.vector.tensor_add(vt[:, 256:384], pv[:, 64:192],
                                 bv_sb[:, 64:192])

        # all of qkv+v runs before attention; S/exp work only starts after,
        # so the ot slots are never contended by pv during the passes
        qkv_order = []
        for c in (0, 512, 1024, 1536):
            for m in (0, 2, 1, 3):
                qkv_order.append(("q", m, c, "s" if (m + c // 512) % 2 else "v"))
        vq = list(range(NT))
        mixed = []
        for j, u in enumerate(qkv_order):
            mixed.append(u)
            if j % 2 == 1 and vq:
                mixed.append(("v", vq.pop(0)))
            if j >= 8 and vq:
                mixed.append(("v", vq.pop(0)))
        for u in mixed:
            if u[0] == "q":
                qk_unit(u[1], u[2], u[3])
            else:
                v_unit(u[1])
        while vq:
            v_unit(vq.pop(0))

        # ---- attention: pass-major, S row-packed pairs ----
        # unit list: per pass p: h01 pairs i=0..4p+3, then h2 pairs
        # (2j, 2j+1). Software pipeline: S(n) | exp(n-1) | O(n-2).
        units = []
        for p in range(NP):
            imax = 4 * (p + 1)
            for i in range(imax):
                units.append(("s01", p, i))
            for j in range(imax // 2):
                units.append(("s2", p, j))
        NU = len(units)                      # 60

        sp_store = {}
        ex_store = {}
        ot_store = {}
        WOFF = [0, 192, 320]                 # v-tile window starts per head
        RSROW = [64, 32, 64]                 # rowsum row per head
        OROW = [0, 64, 0]                    # O partition start per head
        PDST = [(0, 0), (0, 64), (1, 0)]     # (pt index, partition) per head

        def emit_S(n):
            kind, p, a = units[n]
            base = p * PW
            spA = ps.tile([128, 512], f32, tag="sp", bufs=5, name=f"spA{n}")
            spB = ps.tile([128, 512], f32, tag="sp", bufs=5, name=f"spB{n}")
            if kind == "s01":
                i0 = i1 = a
                kt, qt = kAB, qAB
            else:
                i0, i1 = 2 * a, 2 * a + 1
                kt, qt = kC, qC
            lo0 = max(i0 * 128, base)
            lo1 = max(i1 * 128, base)
            for half, (sp, ii, lo) in enumerate(((spA, i0, lo0),
                                                 (spB, i1, lo1))):
                r0, r1 = half * 64, half * 64 + 64
                nc.tensor.matmul(
                    sp[:, lo - base:512],
                    lhsT=kt[r0:r1, ii * 128:(ii + 1) * 128],
                    rhs=qt[r0:r1, lo:base + PW],
                    start=True, stop=True,
                )
            sp_store[n] = (spA, spB, lo0 - base, lo1 - base)

        def emit_E(n):
            kind, p, a = units[n]
            base = p * PW
            spA, spB, offA, offB = sp_store.pop(n)
            ex = sb.tile([128, 1024], bf16, tag="ex", bufs=6, name=f"ex{n}")
            exi = ex.bitcast(i16)
            # the two halves run concurrently on ScalarE and DVE
            if n % 2 == 0:
                nc.scalar.activation(ex[:, offA:512], spA[:, offA:512],
                                     AF.Exp, scale=SCALE)
                nc.vector.tensor_scalar(exi[:, 512 + offB:1024],
                                        spB[:, offB:512], ASCH, BSCH, MUL, ADD)
            else:
                nc.vector.tensor_scalar(exi[:, offA:512], spA[:, offA:512],
                                        ASCH, BSCH, MUL, ADD)
                nc.scalar.activation(ex[:, 512 + offB:1024], spB[:, offB:512],
                                     AF.Exp, scale=SCALE)
            # causal masks on the diagonal 128-blocks (gpsimd)
            i0 = a if kind == "s01" else 2 * a
            if i0 * 128 >= base:
                r = i0 * 128 - base
                shift = 512 if kind == "s01" else 640
                for c0 in (r, r + shift):
                    nc.gpsimd.tensor_mul(ex[:, c0:c0 + 128],
                                         ex[:, c0:c0 + 128], mask_sb[:, :])
            ex_store[n] = (ex, offA, offB)

        def norm(h, p, otx):
            base = p * PW
            rs, orow = RSROW[h], OROW[h]
            pti, prow = PDST[h]
            pdst = pt0 if pti == 0 else pt1
            # rowsum row -> SBUF (ScalarE); PE ones-matmul broadcasts it into
            # a transient 1-bank sp slot; reciprocal + multiply on DVE
            rsb = sb.tile([1, 512], bf16, tag="rsb", bufs=2, name=f"rsb{h}_{p}")
            nc.scalar.activation(rsb[:, :], otx[rs:rs + 1, :], AF.Identity)
            bs = ps.tile([128, 512], f32, tag="sp", bufs=5, name=f"bs{h}_{p}")
            nc.tensor.matmul(bs[:, :], lhsT=ones_sb[:, :], rhs=rsb[:, :],
                             start=True, stop=True)
            rb = sb.tile([64, 512], f32, tag="rb", bufs=2, name=f"rb{h}_{p}")
            nc.vector.reciprocal_approx_fast(rb[:, :], bs[orow:orow + 64, :])
            nc.vector.tensor_mul(pdst[prow:prow + 64, base:base + PW],
                                 otx[orow:orow + 64, :], rb[:, :])

        def get_ot(h, p):
            if (h, p) not in ot_store:
                ot_store[(h, p)] = ps.tile([128, 512], f32, tag="ot", bufs=3,
                                           name=f"ot{h}_{p}")
            return ot_store[(h, p)]

        def emit_O(n):
            kind, p, a = units[n]
            base = p * PW
            imax = 4 * (p + 1)
            ex, offA, offB = ex_store.pop(n)
            if kind == "s01":
                i = a
                lo = max(i * 128, base)
                for half in (0, 1):
                    otx = get_ot(half, p)
                    nc.tensor.matmul(
                        otx[:, lo - base:PW],
                        lhsT=v_sb[i][:, WOFF[half]:WOFF[half] + 128],
                        rhs=ex[:, half * 512 + lo - base:half * 512 + 512],
                        start=(i == 0), stop=(i == imax - 1),
                    )
                if i == imax - 1:
                    norm(0, p, get_ot(0, p))
                    norm(1, p, get_ot(1, p))
                    ot_store.pop((0, p))
                    ot_store.pop((1, p))
            else:
                j = a
                otx = get_ot(2, p)
                for half, ii in enumerate((2 * j, 2 * j + 1)):
                    lo = max(ii * 128, base)
                    nc.tensor.matmul(
                        otx[:, lo - base:PW],
                        lhsT=v_sb[ii][:, WOFF[2]:WOFF[2] + 128],
                        rhs=ex[:, half * 512 + lo - base:half * 512 + 512],
                        start=(ii == 0), stop=(ii == imax - 1),
                    )
                if 2 * j + 1 == imax - 1:
                    norm(2, p, otx)
                    ot_store.pop((2, p))



        def proj_unit(t):
            ysb = sb.tile([128, C], bf16, tag="ysb", bufs=2, name=f"ysb{t}")
            for eng, n0 in (("s", 0), ("v", 384)):
                yp = ps.tile([128, 512], f32, tag="sp", bufs=5,
                             name=f"yp{t}_{n0}")
                nc.tensor.matmul(yp[:, 0:384],
                                 lhsT=pt0[:, t * 128:(t + 1) * 128],
                                 rhs=wpt0_sb[:, n0:n0 + 384],
                                 start=True, stop=False)
                nc.tensor.matmul(yp[:, 0:384],
                                 lhsT=pt1[:, t * 128:(t + 1) * 128],
                                 rhs=wpt1_sb[:, n0:n0 + 384],
                                 start=False, stop=True)
                if eng == "s":
                    nc.scalar.activation(ysb[:, n0:n0 + 384], yp[:, 0:384],
                                         AF.Identity)
                else:
                    nc.vector.tensor_copy(ysb[:, n0:n0 + 384], yp[:, 0:384])
            nc.sync.dma_start(y_d[t * 128:(t + 1) * 128, :], ysb[:, 0:C])

        # pass-end unit indices and proj injection bookkeeping
        pass_end = []
        acc = 0
        for p in range(NP):
            acc += 4 * (p + 1) + 2 * (p + 1)
            pass_end.append(acc - 1)
        proj_queue = []
        proj_ready_at = {pass_end[p] + 3: p for p in range(NP)}
        enqueued = set()

        for n in range(NU + 3):
            if n in proj_ready_at:
                p = proj_ready_at[n]
                proj_queue.extend(range(4 * p, 4 * p + 4))
                enqueued.add(p)
            if n < NU:
                emit_S(n)
            if 1 <= n <= NU:
                emit_E(n - 1)
            if n >= 3:
                emit_O(n - 3)
            # inject at most one proj tile per iteration, spaced out
            if proj_queue and n % 2 == 1:
                proj_unit(proj_queue.pop(0))
        for p in range(NP):
            if p not in enqueued:
                proj_queue.extend(range(4 * p, 4 * p + 4))
        while proj_queue:
            proj_unit(proj_queue.pop(0))

    nc.finalize()
    return nc


def _get_module():
    if "nc" not in _CACHE:
        _CACHE["nc"] = _build_module()
    return _CACHE["nc"]


def make_in_maps(x, w_attn, b_attn, w_proj):
    """Host-side sharding: per-core input dicts (8 cores)."""
    import ml_dtypes
    bf16 = ml_dtypes.bfloat16
    x = np.asarray(x, dtype=np.float32)
    w_attn = np.asarray(w_attn, dtype=np.float32)
    b_attn = np.asarray(b_attn, dtype=np.float32)
    w_proj = np.asarray(w_proj, dtype=np.float32)

    xts = [np.ascontiguousarray(x[b].T).astype(bf16) for b in range(B)]
    mask = np.triu(np.ones((128, 128), np.float32)).astype(bf16)

    in_maps = []
    for c in range(8):
        b = c // G
        hg = c % G
        sl = slice(CPG * hg, CPG * (hg + 1))
        wq = w_attn[0:C][sl]          # [192, 768]
        wk = w_attn[C:2 * C][sl]
        wv = w_attn[2 * C:3 * C][sl]
        # m-tiles: [q01 | q2,q2 | k01 | k2,k2]  -> [768, 512]
        wqkt = np.concatenate([
            wq[0:128], wq[128:192], wq[128:192],
            wk[0:128], wk[128:192], wk[128:192],
        ], axis=0).T
        wqkt = np.ascontiguousarray(wqkt).astype(bf16)
        wvt = np.ascontiguousarray(wv.T).astype(bf16)             # [768, 192]
        bq = b_attn[0:C][sl]
        bk = b_attn[C:2 * C][sl]
        bv = b_attn[2 * C:3 * C][sl]
        bqk = np.zeros((128, 4), np.float32)
        bqk[:, 0] = bq[0:128]
        bqk[:, 1] = np.concatenate([bq[128:192], bq[128:192]])
        bqk[:, 2] = bk[0:128]
        bqk[:, 3] = np.concatenate([bk[128:192], bk[128:192]])
        bvb = np.ascontiguousarray(
            np.broadcast_to(bv, (128, CPG))).astype(np.float32)   # [128, 192]
        wpt = np.ascontiguousarray(w_proj[:, sl].T).astype(bf16)  # [192, 768]
        in_maps.append({
            "xt": xts[b],
            "wqkt": wqkt,
            "wvt": wvt,
            "bqk": bqk,
            "bv": bvb,
            "wpt": wpt,
            "mask": mask,
        })
    return in_maps


def gather(results, b_proj):
    """Sum the 4 head-group partials per batch, add bias."""
    b_proj = np.asarray(b_proj, dtype=np.float32)
    y = np.zeros((B, T, C), np.float32)
    for c in range(8):
        y[c // G] += np.asarray(results[c]["y"], dtype=np.float32)
    y += b_proj
    return y


def run(x, w_attn, b_attn, w_proj, b_proj, trace=False, **kw):
    from concourse.bass_utils import run_bass_kernel_spmd
    nc = _get_module()
    in_maps = make_in_maps(x, w_attn, b_attn, w_proj)
    res = run_bass_kernel_spmd(nc, in_maps, list(range(8)), trace=trace, **kw)
    return gather(res.results, b_proj), res


def kernel(x, w_attn, b_attn, w_proj, b_proj):
    y, _ = run(x, w_attn, b_attn, w_proj, b_proj)
    return y
